# revision 1
# baseline (speedup 1.0000x reference)
"""L2-distance attention (degree-0 DTP block) on 8 Trainium2 NeuronCores.

Sharding: 512 (b,n) nodes split 64 per core -> 1024 edges per core.
Layout: channel-major (feature channels on SBUF partitions, edges on the
free dim). Neighbor/center gathers are one-hot selector matmuls (selectors
built on host from neighbor_indices over the global 512-node space, so the
single SPMD NEFF is core-agnostic). The per-edge radial contraction
kv[o,e] = sum_{r,d} W3[r,o,d]*hdd[r,e]*xe[d,e] runs as a bf16 GEMM against
the Khatri-Rao factor xs[(r,d),e] accumulated over 32 K-chunks in PSUM.
"""
import os
import numpy as np
import ml_dtypes

import concourse.bacc as bacc
import concourse.bass as bass
import concourse.tile as tile
from concourse import mybir
from concourse.bass_utils import run_bass_kernel_spmd

F32 = mybir.dt.float32
F32R = mybir.dt.float32r
BF16 = mybir.dt.bfloat16
AF = mybir.ActivationFunctionType
ALU = mybir.AluOpType

B, N, K, D = 2, 256, 16, 64
H, HID = 4, 128
KVD = 2 * HID
RH = 64
NCORES = 8
NODES = B * N                 # 512
PCORE = NODES // NCORES       # 64 nodes/core
E = PCORE * K                 # 1024 edges/core
SCALE = (HID // H) ** -0.5


def _r(ap):
    return ap


def _emit(nc, tc, P, out, ctx):
    cst = ctx.enter_context(tc.tile_pool(name="cst", bufs=1))
    wk = ctx.enter_context(tc.tile_pool(name="wk", bufs=1))
    lp = ctx.enter_context(tc.tile_pool(name="lp", bufs=3))
    ps = ctx.enter_context(tc.tile_pool(name="ps", bufs=1, space="PSUM"))

    def load(name, dt=F32):
        t = cst.tile(list(P[name].shape), dt, tag=name)
        nc.sync.dma_start(out=t[...], in_=P[name].ap())
        return t

    fT = load("fT"); nsc = load("nsc")
    Wq = load("Wq"); Wxi = load("Wxi")
    WxjI = load("WxjI")
    Sg = load("S", BF16); Cg = load("C", BF16)
    rdT = load("rdT"); M01 = load("M01")
    W1 = load("W1"); b1 = load("b1"); g1 = load("g1")
    W2 = load("W2"); b2 = load("b2"); g2 = load("g2")
    W3t = load("W3sb", BF16); b3T = load("b3T", BF16)
    Wkv = load("Wkv"); Wout = load("Wout")
    selbc = load("selbc", BF16)
    hred = load("hred"); hexp = load("hexp")
    ones64 = load("ones64"); od64 = load("od64"); ones1 = load("ones1x64")

    eps1 = cst.tile([1, 1], F32); nc.vector.memset(eps1[...], 1e-5)

    def pt(tag, p=128, w=512):
        return ps.tile([p, w], F32, tag=tag, name=tag)

    # ---------------- prenorm: xT = fT / max(rms, 1e-12) * norm_scale --------
    sqf = wk.tile([D, NODES], F32)
    nc.scalar.activation(out=sqf[...], in_=fT[...], func=AF.Square)
    ssp = pt("pa", 1)
    nc.tensor.matmul(ssp[:1, :], _r(ones64[...]), _r(sqf[...]), start=True, stop=True)
    rms = wk.tile([1, NODES], F32)
    nc.scalar.activation(out=rms[...], in_=ssp[:1, :NODES], func=AF.Sqrt,
                         scale=1.0 / D)  # sqrt(ss/64) = sqrt(ss)/8
    nc.vector.tensor_scalar_max(out=rms[...], in0=rms[...], scalar1=1e-12)
    rinv = wk.tile([1, NODES], F32)
    nc.vector.reciprocal(out=rinv[...], in_=rms[...])
    rBp = pt("pb", D)
    nc.tensor.matmul(rBp[:D, :], _r(ones1[...]), _r(rinv[...]), start=True, stop=True)
    xT = wk.tile([D, NODES], F32)
    nc.vector.tensor_tensor(out=xT[...], in0=fT[...], in1=rBp[:D, :NODES], op=ALU.mult)
    nc.vector.tensor_scalar_mul(out=xT[...], in0=xT[...], scalar1=nsc[...])

    # ---------- node-major chunks: [x@Wxj | x] via one matmul per chunk ------
    x_nm, xj_nm = [], []
    for ch in range(4):
        pp = pt("pc")
        nc.tensor.matmul(pp[:, :2 * D], _r(xT[:, ch * 128:(ch + 1) * 128]),
                         _r(WxjI[...]), start=True, stop=True)
        xj = wk.tile([128, D], BF16, tag=f"xj{ch}", name=f"xj{ch}")
        nc.scalar.copy(out=xj[...], in_=pp[:, :D])
        xn = wk.tile([128, D], BF16, tag=f"xn{ch}", name=f"xn{ch}")
        nc.scalar.copy(out=xn[...], in_=pp[:, D:2 * D])
        xj_nm.append(xj); x_nm.append(xn)

    # ---------- center replicate: xTe[d, e] = x[ctr(e), d] ----------
    xTe = wk.tile([D, E], F32)
    for nch in range(2):
        pp = pt("pe" if nch == 0 else "pf", D)
        for ch in range(4):
            nc.tensor.matmul(pp[:D, :], x_nm[ch][...],
                             Cg[:, ch, nch * 512:(nch + 1) * 512],
                             start=(ch == 0), stop=(ch == 3))
        nc.scalar.copy(out=xTe[:, nch * 512:(nch + 1) * 512], in_=pp[:D, :])

    # ---------- edge features: xeT = xg(neighbor) + xi(center) ----------
    xeT_ps = []
    for nch in range(2):
        pp = pt("pa" if nch == 0 else "pb", D)
        xeT_ps.append(pp)
        for ch in range(4):
            nc.tensor.matmul(pp[:D, :], xj_nm[ch][...],
                             Sg[:, ch, nch * 512:(nch + 1) * 512],
                             start=(ch == 0), stop=False)
        nc.tensor.matmul(pp[:D, :], _r(Wxi[...]),
                         _r(xTe[:, nch * 512:(nch + 1) * 512]),
                         start=False, stop=True)
    stack = wk.tile([128, E], BF16)   # [xeT; xeT] bf16
    for nch in range(2):
        sl = slice(nch * 512, (nch + 1) * 512)
        nc.vector.tensor_copy(out=stack[:D, sl], in_=xeT_ps[nch][:D, :])
        nc.scalar.copy(out=stack[D:, sl], in_=xeT_ps[nch][:D, :])

    # ---------- queries per edge ----------
    qTe = wk.tile([HID, E], F32)
    for nch in range(2):
        pp = pt("pc")
        nc.tensor.matmul(pp[...], _r(Wq[...]), _r(xTe[:, nch * 512:(nch + 1) * 512]),
                         start=True, stop=True)
        nc.scalar.copy(out=qTe[:, nch * 512:(nch + 1) * 512], in_=pp[...])

    # ---------- radial MLP: 2 x (linear -> silu -> LN*g), channel-major ------
    def radial_layer(z_src_ps, bias, g, out_dt, tg):
        z = wk.tile([RH, E], F32, tag=tg + "z", name=tg + "z")
        for nch in range(2):
            nc.scalar.activation(out=z[:, nch * 512:(nch + 1) * 512],
                                 in_=z_src_ps[nch][:RH, :], func=AF.Silu,
                                 bias=bias[...], scale=1.0)
        sq = wk.tile([RH, E], F32, tag=tg + "q", name=tg + "q")
        nc.scalar.activation(out=sq[...], in_=z[...], func=AF.Square)
        s1 = wk.tile([1, E], F32, tag=tg + "s1", name=tg + "s1")
        s2 = wk.tile([1, E], F32, tag=tg + "s2", name=tg + "s2")
        for nch in range(2):
            sl = slice(nch * 512, (nch + 1) * 512)
            p1 = pt("pc", 1)
            nc.tensor.matmul(p1[:1, :], _r(ones64[...]), _r(z[:, sl]), start=True, stop=True)
            nc.scalar.copy(out=s1[:, sl], in_=p1[:1, :])
            p2 = pt("pd", 1)
            nc.tensor.matmul(p2[:1, :], _r(ones64[...]), _r(sq[:, sl]), start=True, stop=True)
            nc.scalar.copy(out=s2[:, sl], in_=p2[:1, :])
        m2 = wk.tile([1, E], F32, tag=tg + "m2", name=tg + "m2")
        nc.vector.scalar_tensor_tensor(out=m2[...], in0=s1[...], scalar=1.0 / RH,
                                       in1=s1[...], op0=ALU.mult, op1=ALU.mult)
        v64 = wk.tile([1, E], F32, tag=tg + "v", name=tg + "v")   # 64*var = s2 - s1^2/64
        nc.vector.scalar_tensor_tensor(out=v64[...], in0=m2[...], scalar=-1.0,
                                       in1=s2[...], op0=ALU.mult, op1=ALU.add)
        sd = wk.tile([1, E], F32, tag=tg + "sd", name=tg + "sd")
        nc.scalar.activation(out=sd[...], in_=v64[...], func=AF.Sqrt,
                             bias=eps1[...], scale=1.0 / RH)  # sqrt(var+eps)
        rstd = wk.tile([1, E], F32, tag=tg + "rs", name=tg + "rs")
        nc.vector.reciprocal(out=rstd[...], in_=sd[...])
        hddo = wk.tile([RH, E], out_dt, tag=tg)
        for nch in range(2):
            sl = slice(nch * 512, (nch + 1) * 512)
            muB = pt("pc", RH)
            nc.tensor.matmul(muB[:RH, :], _r(od64[...]), _r(s1[:, sl]), start=True, stop=True)
            rsB = pt("pd", RH)
            nc.tensor.matmul(rsB[:RH, :], _r(ones1[...]), _r(rstd[:, sl]), start=True, stop=True)
            d1 = wk.tile([RH, 512], F32, tag=tg + "d1", name=tg + "d1")
            nc.vector.tensor_tensor(out=d1[...], in0=z[:, sl], in1=muB[:RH, :], op=ALU.subtract)
            d2 = wk.tile([RH, 512], F32, tag=tg + "d2", name=tg + "d2")
            nc.vector.tensor_tensor(out=d2[...], in0=d1[...], in1=rsB[:RH, :], op=ALU.mult)
            nc.vector.tensor_scalar_mul(out=hddo[:, sl], in0=d2[...], scalar1=g[...])
        return hddo

    h1ps = []
    for nch in range(2):
        pp = pt("pe" if nch == 0 else "pf", RH)
        nc.tensor.matmul(pp[:RH, :], _r(W1[...]), _r(rdT[:, nch * 512:(nch + 1) * 512]),
                         start=True, stop=True)
        h1ps.append(pp)
    hdd1 = radial_layer(h1ps, b1, g1, F32, "h1")
    h2ps = []
    for nch in range(2):
        pp = pt("pe" if nch == 0 else "pf", RH)
        nc.tensor.matmul(pp[:RH, :], _r(W2[...]), _r(hdd1[:, nch * 512:(nch + 1) * 512]),
                         start=True, stop=True)
        h2ps.append(pp)
    hddT = radial_layer(h2ps, b2, g2, BF16, "h2")

    # ---------- big GEMM: kv[o,e] = sum_{rd} W3'[rd,o] * xs[rd,e] ----------
    kvtags = ["pa", "pb", "pc", "pd"]
    kvps = [[pt(kvtags[2 * m + n]) for n in range(2)] for m in range(2)]
    for c in range(32):
        hBp = [pt("pe"), pt("pf")]
        for nch in range(2):
            nc.tensor.matmul(hBp[nch][...], selbc[:, c, :],
                             hddT[:, nch * 512:(nch + 1) * 512],
                             start=True, stop=True)
        hBs = lp.tile([128, E], BF16, tag="hBs", name="hBs")
        for nch in range(2):
            nc.scalar.copy(out=hBs[:, nch * 512:(nch + 1) * 512], in_=hBp[nch][...])
        xs = lp.tile([128, E], BF16, tag="xs", name="xs")
        nc.vector.tensor_tensor(out=xs[...], in0=stack[...], in1=hBs[...], op=ALU.mult)
        for m in range(2):
            for nch in range(2):
                nc.tensor.matmul(kvps[m][nch][...],
                                 W3t[:, c, m * 128:(m + 1) * 128],
                                 xs[:, nch * 512:(nch + 1) * 512],
                                 start=(c == 0), stop=False)
    for m in range(2):
        for nch in range(2):
            nc.tensor.matmul(kvps[m][nch][...], b3T[:, m * 128:(m + 1) * 128],
                             stack[:D, nch * 512:(nch + 1) * 512],
                             start=False, stop=True)
    kvT = wk.tile([128, 2, E], F32)
    for m in range(2):
        for nch in range(2):
            nc.scalar.copy(out=kvT[:, m, nch * 512:(nch + 1) * 512],
                           in_=kvps[m][nch][...])

    # ---------- kv2 = Wkv^T @ kv : kk rows 0:128, vv rows 128:256 ----------
    kkT = wk.tile([HID, E], F32)
    vvT = wk.tile([HID, E], F32)
    for m, dst_t in ((0, kkT), (1, vvT)):
        for nch in range(2):
            pp = pt("pa" if nch == 0 else "pb")
            for kc in range(2):
                nc.tensor.matmul(pp[...],
                                 _r(Wkv[:, kc, m * 128:(m + 1) * 128]),
                                 _r(kvT[:, kc, nch * 512:(nch + 1) * 512]),
                                 start=(kc == 0), stop=(kc == 1))
            nc.scalar.copy(out=dst_t[:, nch * 512:(nch + 1) * 512], in_=pp[...])

    # ---------- attention ----------
    dif = wk.tile([HID, E], F32)
    nc.vector.scalar_tensor_tensor(out=dif[...], in0=qTe[...], scalar=1e-6,
                                   in1=kkT[...], op0=ALU.add, op1=ALU.subtract)
    sqd = wk.tile([HID, E], F32)
    nc.scalar.activation(out=sqd[...], in_=dif[...], func=AF.Square)
    Pm = wk.tile([H, E], F32)
    for nch in range(2):
        sl = slice(nch * 512, (nch + 1) * 512)
        pp = pt("pc", H)
        nc.tensor.matmul(pp[:H, :], _r(hred[...]), _r(sqd[:, sl]), start=True, stop=True)
        sdt = wk.tile([H, 512], F32, tag="sdt", name="sdt")
        nc.scalar.activation(out=sdt[...], in_=pp[:H, :], func=AF.Sqrt)
        pe_ = wk.tile([H, 512], F32, tag="pe_", name="pe_")
        nc.scalar.activation(out=pe_[...], in_=sdt[...], func=AF.Exp, scale=-SCALE)
        nc.vector.tensor_tensor(out=Pm[:, sl], in0=pe_[...], in1=M01[:, sl], op=ALU.mult)
    Ssum = wk.tile([H, PCORE], F32)
    nc.vector.tensor_reduce(out=Ssum[...],
                            in_=Pm[...].rearrange("h (j k) -> h j k", k=K),
                            axis=mybir.AxisListType.X, op=ALU.add)
    Rinv = wk.tile([H, PCORE], F32)
    nc.vector.reciprocal(out=Rinv[...], in_=Ssum[...])
    ow = wk.tile([HID, PCORE], F32)
    for nch in range(2):
        sl = slice(nch * 512, (nch + 1) * 512)
        pp = pt("pd")
        nc.tensor.matmul(pp[...], _r(hexp[...]), _r(Pm[:, sl]), start=True, stop=True)
        wv = wk.tile([HID, 512], F32, tag="wv", name="wv")
        nc.vector.tensor_tensor(out=wv[...], in0=pp[...], in1=vvT[:, sl], op=ALU.mult)
        nc.vector.tensor_reduce(out=ow[:, nch * 32:(nch + 1) * 32],
                                in_=wv[...].rearrange("c (j k) -> c j k", k=K),
                                axis=mybir.AxisListType.X, op=ALU.add)
    rfp = pt("pc")
    nc.tensor.matmul(rfp[:, :PCORE], _r(hexp[...]), _r(Rinv[...]), start=True, stop=True)
    oT = wk.tile([HID, PCORE], F32)
    nc.vector.tensor_tensor(out=oT[...], in0=ow[...], in1=rfp[:, :PCORE], op=ALU.mult)
    ofp = pt("pd")
    nc.tensor.matmul(ofp[:D, :PCORE], _r(Wout[...]), _r(oT[...]), start=True, stop=True)
    outFT = wk.tile([D, PCORE], F32)
    nc.scalar.copy(out=outFT[...], in_=ofp[:D, :PCORE])
    dst = bass.AP(tensor=out, offset=0, ap=[[1, D], [D, PCORE]])
    nc.sync.dma_start(out=dst, in_=outFT[...])


def _build_nc():
    nc = bacc.Bacc("TRN2", target_bir_lowering=False, debug=False,
                   num_devices=NCORES)
    P = {}
    def inp(name, shape, dt=F32):
        P[name] = nc.declare_dram_parameter(name, list(shape), dt, isOutput=False)
    inp("fT", (D, NODES)); inp("nsc", (D, 1))
    inp("Wq", (D, HID)); inp("Wxi", (D, D)); inp("WxjI", (D, 2 * D))
    inp("S", (128, 4, E), BF16); inp("C", (128, 4, E), BF16)
    inp("rdT", (1, E)); inp("M01", (H, E))
    inp("W1", (1, RH)); inp("b1", (RH, 1)); inp("g1", (RH, 1))
    inp("W2", (RH, RH)); inp("b2", (RH, 1)); inp("g2", (RH, 1))
    inp("W3sb", (128, 32, KVD), BF16); inp("b3T", (D, KVD), BF16)
    inp("Wkv", (128, 2, KVD)); inp("Wout", (HID, D))
    inp("selbc", (RH, 32, 128), BF16)
    inp("hred", (128, H)); inp("hexp", (H, 128))
    inp("ones64", (D, 1)); inp("od64", (1, D)); inp("ones1x64", (1, D))
    out = nc.declare_dram_parameter("out", [PCORE, D], F32, isOutput=True)
    import contextlib
    with tile.TileContext(nc) as tc:
        with contextlib.ExitStack() as ctx:
            _emit(nc, tc, P, out, ctx)
    nc.finalize()
    return nc


_NC = None


def kernel(features, neighbor_indices, neighbor_mask, rel_dist, norm_scale,
           Wq, Wxi, Wxj, rp_W1, rp_b1, rp_g1, rp_W2, rp_b2, rp_g2,
           rp_W3, rp_b3, Wkv_out, Wout):
    global _NC
    bf = ml_dtypes.bfloat16
    f = np.asarray(features, np.float32)
    idx = np.asarray(neighbor_indices).astype(np.int64)
    msk = np.asarray(neighbor_mask).astype(np.float32)
    rd = np.asarray(rel_dist, np.float32)

    fT = np.ascontiguousarray(f[..., 0].reshape(NODES, D).T)
    WxjI = np.concatenate([np.asarray(Wxj, np.float32),
                           np.eye(D, dtype=np.float32)], axis=1)
    W3sb = np.ascontiguousarray(
        np.asarray(rp_W3, np.float32)
        .reshape(RH, KVD, D).transpose(0, 2, 1)       # (r, d, o)
        .reshape(RH * D, KVD)                         # row = r*64 + d
        .reshape(32, 128, KVD).transpose(1, 0, 2)     # (p, chunk, o)
    ).astype(bf)
    b3T = np.ascontiguousarray(
        np.asarray(rp_b3, np.float32).reshape(KVD, D).T).astype(bf)
    WkvP = np.ascontiguousarray(
        np.asarray(Wkv_out, np.float32).reshape(2, 128, KVD).transpose(1, 0, 2))
    selbc = np.zeros((RH, 32, 128), bf)
    for c in range(32):
        selbc[2 * c, c, :64] = 1
        selbc[2 * c + 1, c, 64:] = 1
    hred = np.zeros((128, H), np.float32)
    for h in range(H):
        hred[h * 32:(h + 1) * 32, h] = 1
    hexp = np.ascontiguousarray(hred.T)

    shared = dict(
        fT=fT, nsc=np.asarray(norm_scale, np.float32).reshape(D, 1),
        Wq=np.asarray(Wq, np.float32), Wxi=np.asarray(Wxi, np.float32),
        WxjI=WxjI,
        W1=np.asarray(rp_W1, np.float32).reshape(1, RH),
        b1=np.asarray(rp_b1, np.float32).reshape(RH, 1),
        g1=np.asarray(rp_g1, np.float32).reshape(RH, 1),
        W2=np.asarray(rp_W2, np.float32),
        b2=np.asarray(rp_b2, np.float32).reshape(RH, 1),
        g2=np.asarray(rp_g2, np.float32).reshape(RH, 1),
        W3sb=W3sb, b3T=b3T, Wkv=WkvP, Wout=np.asarray(Wout, np.float32),
        selbc=selbc, hred=hred, hexp=hexp,
        ones64=np.ones((D, 1), np.float32),
        od64=np.full((1, D), 1.0 / RH, np.float32),
        ones1x64=np.ones((1, D), np.float32),
    )

    in_maps = []
    for c in range(NCORES):
        b = (c * PCORE) // N
        loc_n = np.arange(c * PCORE, (c + 1) * PCORE) - b * N
        nb = idx[b, loc_n, :].reshape(E)
        gctr = b * N + np.repeat(loc_n, K)
        gnbr = b * N + nb
        S = np.zeros((4, 128, E), bf)
        S[gnbr // 128, gnbr % 128, np.arange(E)] = 1
        C = np.zeros((4, 128, E), bf)
        C[gctr // 128, gctr % 128, np.arange(E)] = 1
        m = dict(shared)
        m.update(S=np.ascontiguousarray(S.transpose(1, 0, 2)),
                 C=np.ascontiguousarray(C.transpose(1, 0, 2)),
                 rdT=rd[b, loc_n, :, 0].reshape(1, E).astype(np.float32),
                 M01=np.broadcast_to(msk[b, loc_n, :].reshape(1, E),
                                     (H, E)).astype(np.float32).copy())
        in_maps.append(m)

    if _NC is None:
        _NC = _build_nc()
    res = run_bass_kernel_spmd(_NC, in_maps, list(range(NCORES)))
    full = np.concatenate([res.results[c]["out"] for c in range(NCORES)], axis=0)
    return full.reshape(B, N, D, 1).astype(np.float32)



# revision 2
# speedup vs baseline: 13.7199x; 13.7199x over previous
"""L2-distance attention (degree-0 DTP block) on 8 Trainium2 NeuronCores.

Sharding: 512 (b,n) nodes split 64 per core -> 1024 edges per core; cores
0-3 serve batch 0, cores 4-7 batch 1, so each core only needs its batch's
256 node features. Layout: channel-major (feature channels on SBUF
partitions, edges on the free dim). Neighbor/center gathers are one-hot
selector matmuls; the selectors are built ON DEVICE from the uploaded
index rows (partition-broadcast matmul + is_equal against an iota column),
so the per-call host->device traffic is just features + indices + mask +
distances (~0.6 MB). All weight-derived operands are device-resident and
cached across calls keyed on a content hash; the jit executable is built
once. The per-edge radial contraction kv[o,e] = sum_{r,d} W3[r,o,d]*
hdd[r,e]*xe[d,e] runs as a bf16 GEMM against the Khatri-Rao factor
xs[(r,d),e] accumulated over 32 K-chunks in PSUM.
"""
import hashlib
import numpy as np
import ml_dtypes

import jax
from jax.sharding import Mesh, PartitionSpec, NamedSharding
from jax.experimental.shard_map import shard_map

import concourse.bacc as bacc
import concourse.bass as bass
import concourse.tile as tile
from concourse import mybir, bass2jax

F32 = mybir.dt.float32
BF16 = mybir.dt.bfloat16
AF = mybir.ActivationFunctionType
ALU = mybir.AluOpType

B, N, K, D = 2, 256, 16, 64
H, HID = 4, 128
KVD = 2 * HID
RH = 64
NCORES = 8
NODES = B * N                 # 512
PCORE = NODES // NCORES       # 64 nodes/core
E = PCORE * K                 # 1024 edges/core
NB = N                        # per-batch node space a core needs (256)
NCH = NB // 128               # 2 selector chunks
SCALE = (HID // H) ** -0.5


def _emit(nc, tc, P, out, ctx):
    cst = ctx.enter_context(tc.tile_pool(name="cst", bufs=1))
    wk = ctx.enter_context(tc.tile_pool(name="wk", bufs=1))
    lp = ctx.enter_context(tc.tile_pool(name="lp", bufs=3))
    ps = ctx.enter_context(tc.tile_pool(name="ps", bufs=1, space="PSUM"))

    def load(name, dt=F32):
        t = cst.tile(list(P[name].shape), dt, tag=name)
        nc.sync.dma_start(out=t[...], in_=P[name].ap())
        return t

    fT = load("fT"); nsc = load("nsc")
    Wq = load("Wq"); Wxi = load("Wxi")
    WxjI = load("WxjI")
    gnb = load("gnb"); gcb = load("gcb")
    rdT = load("rdT"); M1 = load("M1")
    W1 = load("W1"); b1 = load("b1"); g1 = load("g1")
    W2 = load("W2"); b2 = load("b2"); g2 = load("g2")
    W3t = load("W3sb", BF16); b3T = load("b3T", BF16)
    Wkv = load("Wkv"); Wout = load("Wout")
    selbc = load("selbc", BF16)
    hred = load("hred"); hexp = load("hexp")
    ones64 = load("ones64"); od64 = load("od64"); ones1 = load("ones1x64")
    ones1x128 = load("ones1x128"); ones1x4 = load("ones1x4")
    iota2 = load("iota2")

    eps1 = cst.tile([1, 1], F32); nc.vector.memset(eps1[...], 1e-5)

    def pt(tag, p=128, w=512):
        return ps.tile([p, w], F32, tag=tag, name=tag)

    # ---------------- selectors built on device from index rows -------------
    # Sg[p, ch, e] = 1 iff neighbor(e) == p + 128*ch   (bf16 one-hot)
    Sg = wk.tile([128, NCH, E], BF16)
    Cg = wk.tile([128, NCH, E], BF16)
    for src, dstT, tga, tgb in ((gnb, Sg, "pa", "pb"), (gcb, Cg, "pc", "pd")):
        for nch in range(2):
            sl = slice(nch * 512, (nch + 1) * 512)
            pp = pt(tga if nch == 0 else tgb)
            nc.tensor.matmul(pp[...], ones1x128[...], src[:, sl],
                             start=True, stop=True)
            for ch in range(NCH):
                nc.vector.tensor_scalar(dstT[:, ch, sl], pp[...],
                                        iota2[:, ch:ch + 1], None, ALU.is_equal)

    # ---------------- prenorm: xT = fT / max(rms, 1e-12) * norm_scale --------
    sqf = wk.tile([D, NB], F32)
    nc.scalar.activation(out=sqf[...], in_=fT[...], func=AF.Square)
    ssp = pt("pe", 1)
    nc.tensor.matmul(ssp[:1, :NB], ones64[...], sqf[...], start=True, stop=True)
    rms = wk.tile([1, NB], F32)
    nc.scalar.activation(out=rms[...], in_=ssp[:1, :NB], func=AF.Sqrt,
                         scale=1.0 / D)  # sqrt(ss/64) = sqrt(ss)/8
    nc.vector.tensor_scalar_max(out=rms[...], in0=rms[...], scalar1=1e-12)
    rinv = wk.tile([1, NB], F32)
    nc.vector.reciprocal(out=rinv[...], in_=rms[...])
    rBp = pt("pf", D)
    nc.tensor.matmul(rBp[:D, :NB], ones1[...], rinv[...], start=True, stop=True)
    xT = wk.tile([D, NB], F32)
    nc.vector.tensor_tensor(out=xT[...], in0=fT[...], in1=rBp[:D, :NB], op=ALU.mult)
    nc.vector.tensor_scalar_mul(out=xT[...], in0=xT[...], scalar1=nsc[...])

    # ---------- node-major chunks: [x@Wxj | x] via one matmul per chunk ------
    x_nm, xj_nm = [], []
    for ch in range(NCH):
        pp = pt("pe")
        nc.tensor.matmul(pp[:, :2 * D], xT[:, ch * 128:(ch + 1) * 128],
                         WxjI[...], start=True, stop=True)
        xj = wk.tile([128, D], BF16, tag=f"xj{ch}", name=f"xj{ch}")
        nc.scalar.copy(out=xj[...], in_=pp[:, :D])
        xn = wk.tile([128, D], BF16, tag=f"xn{ch}", name=f"xn{ch}")
        nc.scalar.copy(out=xn[...], in_=pp[:, D:2 * D])
        xj_nm.append(xj); x_nm.append(xn)

    # ---------- center replicate: xTe[d, e] = x[ctr(e), d] ----------
    xTe = wk.tile([D, E], F32)
    for nch in range(2):
        pp = pt("pe" if nch == 0 else "pf", D)
        for ch in range(NCH):
            nc.tensor.matmul(pp[:D, :], x_nm[ch][...],
                             Cg[:, ch, nch * 512:(nch + 1) * 512],
                             start=(ch == 0), stop=(ch == NCH - 1))
        nc.scalar.copy(out=xTe[:, nch * 512:(nch + 1) * 512], in_=pp[:D, :])

    # ---------- edge features: xeT = xg(neighbor) + xi(center) ----------
    xeT_ps = []
    for nch in range(2):
        pp = pt("pa" if nch == 0 else "pb", D)
        xeT_ps.append(pp)
        for ch in range(NCH):
            nc.tensor.matmul(pp[:D, :], xj_nm[ch][...],
                             Sg[:, ch, nch * 512:(nch + 1) * 512],
                             start=(ch == 0), stop=False)
        nc.tensor.matmul(pp[:D, :], Wxi[...],
                         xTe[:, nch * 512:(nch + 1) * 512],
                         start=False, stop=True)
    stack = wk.tile([128, E], BF16)   # [xeT; xeT] bf16
    for nch in range(2):
        sl = slice(nch * 512, (nch + 1) * 512)
        nc.vector.tensor_copy(out=stack[:D, sl], in_=xeT_ps[nch][:D, :])
        nc.scalar.copy(out=stack[D:, sl], in_=xeT_ps[nch][:D, :])

    # ---------- queries per edge ----------
    qTe = wk.tile([HID, E], F32)
    for nch in range(2):
        pp = pt("pc")
        nc.tensor.matmul(pp[...], Wq[...], xTe[:, nch * 512:(nch + 1) * 512],
                         start=True, stop=True)
        nc.scalar.copy(out=qTe[:, nch * 512:(nch + 1) * 512], in_=pp[...])

    # ---------- radial MLP: 2 x (linear -> silu -> LN*g), channel-major ------
    def radial_layer(z_src_ps, bias, g, out_dt, tg):
        z = wk.tile([RH, E], F32, tag=tg + "z", name=tg + "z")
        for nch in range(2):
            nc.scalar.activation(out=z[:, nch * 512:(nch + 1) * 512],
                                 in_=z_src_ps[nch][:RH, :], func=AF.Silu,
                                 bias=bias[...], scale=1.0)
        sq = wk.tile([RH, E], F32, tag=tg + "q", name=tg + "q")
        nc.scalar.activation(out=sq[...], in_=z[...], func=AF.Square)
        s1 = wk.tile([1, E], F32, tag=tg + "s1", name=tg + "s1")
        s2 = wk.tile([1, E], F32, tag=tg + "s2", name=tg + "s2")
        for nch in range(2):
            sl = slice(nch * 512, (nch + 1) * 512)
            p1 = pt("pc", 1)
            nc.tensor.matmul(p1[:1, :], ones64[...], z[:, sl], start=True, stop=True)
            nc.scalar.copy(out=s1[:, sl], in_=p1[:1, :])
            p2 = pt("pd", 1)
            nc.tensor.matmul(p2[:1, :], ones64[...], sq[:, sl], start=True, stop=True)
            nc.scalar.copy(out=s2[:, sl], in_=p2[:1, :])
        m2 = wk.tile([1, E], F32, tag=tg + "m2", name=tg + "m2")
        nc.vector.scalar_tensor_tensor(out=m2[...], in0=s1[...], scalar=1.0 / RH,
                                       in1=s1[...], op0=ALU.mult, op1=ALU.mult)
        v64 = wk.tile([1, E], F32, tag=tg + "v", name=tg + "v")   # 64*var = s2 - s1^2/64
        nc.vector.scalar_tensor_tensor(out=v64[...], in0=m2[...], scalar=-1.0,
                                       in1=s2[...], op0=ALU.mult, op1=ALU.add)
        sd = wk.tile([1, E], F32, tag=tg + "sd", name=tg + "sd")
        nc.scalar.activation(out=sd[...], in_=v64[...], func=AF.Sqrt,
                             bias=eps1[...], scale=1.0 / RH)  # sqrt(var+eps)
        rstd = wk.tile([1, E], F32, tag=tg + "rs", name=tg + "rs")
        nc.vector.reciprocal(out=rstd[...], in_=sd[...])
        hddo = wk.tile([RH, E], out_dt, tag=tg)
        for nch in range(2):
            sl = slice(nch * 512, (nch + 1) * 512)
            muB = pt("pc", RH)
            nc.tensor.matmul(muB[:RH, :], od64[...], s1[:, sl], start=True, stop=True)
            rsB = pt("pd", RH)
            nc.tensor.matmul(rsB[:RH, :], ones1[...], rstd[:, sl], start=True, stop=True)
            d1 = wk.tile([RH, 512], F32, tag=tg + "d1", name=tg + "d1")
            nc.vector.tensor_tensor(out=d1[...], in0=z[:, sl], in1=muB[:RH, :], op=ALU.subtract)
            d2 = wk.tile([RH, 512], F32, tag=tg + "d2", name=tg + "d2")
            nc.vector.tensor_tensor(out=d2[...], in0=d1[...], in1=rsB[:RH, :], op=ALU.mult)
            nc.vector.tensor_scalar_mul(out=hddo[:, sl], in0=d2[...], scalar1=g[...])
        return hddo

    h1ps = []
    for nch in range(2):
        pp = pt("pe" if nch == 0 else "pf", RH)
        nc.tensor.matmul(pp[:RH, :], W1[...], rdT[:, nch * 512:(nch + 1) * 512],
                         start=True, stop=True)
        h1ps.append(pp)
    hdd1 = radial_layer(h1ps, b1, g1, F32, "h1")
    h2ps = []
    for nch in range(2):
        pp = pt("pe" if nch == 0 else "pf", RH)
        nc.tensor.matmul(pp[:RH, :], W2[...], hdd1[:, nch * 512:(nch + 1) * 512],
                         start=True, stop=True)
        h2ps.append(pp)
    hddT = radial_layer(h2ps, b2, g2, BF16, "h2")

    # ---------- big GEMM: kv[o,e] = sum_{rd} W3'[rd,o] * xs[rd,e] ----------
    kvtags = ["pa", "pb", "pc", "pd"]
    kvps = [[pt(kvtags[2 * m + n]) for n in range(2)] for m in range(2)]
    for c in range(32):
        hBp = [pt("pe"), pt("pf")]
        for nch in range(2):
            nc.tensor.matmul(hBp[nch][...], selbc[:, c, :],
                             hddT[:, nch * 512:(nch + 1) * 512],
                             start=True, stop=True)
        hBs = lp.tile([128, E], BF16, tag="hBs", name="hBs")
        for nch in range(2):
            nc.scalar.copy(out=hBs[:, nch * 512:(nch + 1) * 512], in_=hBp[nch][...])
        xs = lp.tile([128, E], BF16, tag="xs", name="xs")
        nc.vector.tensor_tensor(out=xs[...], in0=stack[...], in1=hBs[...], op=ALU.mult)
        for m in range(2):
            for nch in range(2):
                nc.tensor.matmul(kvps[m][nch][...],
                                 W3t[:, c, m * 128:(m + 1) * 128],
                                 xs[:, nch * 512:(nch + 1) * 512],
                                 start=(c == 0), stop=False)
    for m in range(2):
        for nch in range(2):
            nc.tensor.matmul(kvps[m][nch][...], b3T[:, m * 128:(m + 1) * 128],
                             stack[:D, nch * 512:(nch + 1) * 512],
                             start=False, stop=True)
    kvT = wk.tile([128, 2, E], F32)
    for m in range(2):
        for nch in range(2):
            nc.scalar.copy(out=kvT[:, m, nch * 512:(nch + 1) * 512],
                           in_=kvps[m][nch][...])

    # ---------- kv2 = Wkv^T @ kv : kk rows 0:128, vv rows 128:256 ----------
    kkT = wk.tile([HID, E], F32)
    vvT = wk.tile([HID, E], F32)
    for m, dst_t in ((0, kkT), (1, vvT)):
        for nch in range(2):
            pp = pt("pa" if nch == 0 else "pb")
            for kc in range(2):
                nc.tensor.matmul(pp[...],
                                 Wkv[:, kc, m * 128:(m + 1) * 128],
                                 kvT[:, kc, nch * 512:(nch + 1) * 512],
                                 start=(kc == 0), stop=(kc == 1))
            nc.scalar.copy(out=dst_t[:, nch * 512:(nch + 1) * 512], in_=pp[...])

    # ---------- attention ----------
    dif = wk.tile([HID, E], F32)
    nc.vector.scalar_tensor_tensor(out=dif[...], in0=qTe[...], scalar=1e-6,
                                   in1=kkT[...], op0=ALU.add, op1=ALU.subtract)
    sqd = wk.tile([HID, E], F32)
    nc.scalar.activation(out=sqd[...], in_=dif[...], func=AF.Square)
    Pm = wk.tile([H, E], F32)
    for nch in range(2):
        sl = slice(nch * 512, (nch + 1) * 512)
        pp = pt("pc", H)
        nc.tensor.matmul(pp[:H, :], hred[...], sqd[:, sl], start=True, stop=True)
        pm4 = pt("pe", H)
        nc.tensor.matmul(pm4[:H, :], ones1x4[...], M1[:, sl], start=True, stop=True)
        sdt = wk.tile([H, 512], F32, tag="sdt", name="sdt")
        nc.scalar.activation(out=sdt[...], in_=pp[:H, :], func=AF.Sqrt)
        pe_ = wk.tile([H, 512], F32, tag="pe_", name="pe_")
        nc.scalar.activation(out=pe_[...], in_=sdt[...], func=AF.Exp, scale=-SCALE)
        nc.vector.tensor_tensor(out=Pm[:, sl], in0=pe_[...], in1=pm4[:H, :], op=ALU.mult)
    Ssum = wk.tile([H, PCORE], F32)
    nc.vector.tensor_reduce(out=Ssum[...],
                            in_=Pm[...].rearrange("h (j k) -> h j k", k=K),
                            axis=mybir.AxisListType.X, op=ALU.add)
    Rinv = wk.tile([H, PCORE], F32)
    nc.vector.reciprocal(out=Rinv[...], in_=Ssum[...])
    ow = wk.tile([HID, PCORE], F32)
    for nch in range(2):
        sl = slice(nch * 512, (nch + 1) * 512)
        pp = pt("pd")
        nc.tensor.matmul(pp[...], hexp[...], Pm[:, sl], start=True, stop=True)
        wv = wk.tile([HID, 512], F32, tag="wv", name="wv")
        nc.vector.tensor_tensor(out=wv[...], in0=pp[...], in1=vvT[:, sl], op=ALU.mult)
        nc.vector.tensor_reduce(out=ow[:, nch * 32:(nch + 1) * 32],
                                in_=wv[...].rearrange("c (j k) -> c j k", k=K),
                                axis=mybir.AxisListType.X, op=ALU.add)
    rfp = pt("pc")
    nc.tensor.matmul(rfp[:, :PCORE], hexp[...], Rinv[...], start=True, stop=True)
    oT = wk.tile([HID, PCORE], F32)
    nc.vector.tensor_tensor(out=oT[...], in0=ow[...], in1=rfp[:, :PCORE], op=ALU.mult)
    ofp = pt("pd")
    nc.tensor.matmul(ofp[:D, :PCORE], Wout[...], oT[...], start=True, stop=True)
    outFT = wk.tile([D, PCORE], F32)
    nc.scalar.copy(out=outFT[...], in_=ofp[:D, :PCORE])
    dst = bass.AP(tensor=out, offset=0, ap=[[1, D], [D, PCORE]])
    nc.sync.dma_start(out=dst, in_=outFT[...])


def _build_nc():
    nc = bacc.Bacc("TRN2", target_bir_lowering=False, debug=False,
                   num_devices=NCORES)
    P = {}
    def inp(name, shape, dt=F32):
        P[name] = nc.declare_dram_parameter(name, list(shape), dt, isOutput=False)
    inp("fT", (D, NB)); inp("nsc", (D, 1))
    inp("Wq", (D, HID)); inp("Wxi", (D, D)); inp("WxjI", (D, 2 * D))
    inp("gnb", (1, E)); inp("gcb", (1, E))
    inp("rdT", (1, E)); inp("M1", (1, E))
    inp("W1", (1, RH)); inp("b1", (RH, 1)); inp("g1", (RH, 1))
    inp("W2", (RH, RH)); inp("b2", (RH, 1)); inp("g2", (RH, 1))
    inp("W3sb", (128, 32, KVD), BF16); inp("b3T", (D, KVD), BF16)
    inp("Wkv", (128, 2, KVD)); inp("Wout", (HID, D))
    inp("selbc", (RH, 32, 128), BF16)
    inp("hred", (128, H)); inp("hexp", (H, 128))
    inp("ones64", (D, 1)); inp("od64", (1, D)); inp("ones1x64", (1, D))
    inp("ones1x128", (1, 128)); inp("ones1x4", (1, H))
    inp("iota2", (128, NCH))
    out = nc.declare_dram_parameter("out", [PCORE, D], F32, isOutput=True)
    import contextlib
    with tile.TileContext(nc) as tc:
        with contextlib.ExitStack() as ctx:
            _emit(nc, tc, P, out, ctx)
    nc.finalize()
    return nc


class _Runner:
    """Builds the sharded jit once; subsequent calls reuse the executable."""

    def __init__(self, nc):
        bass2jax.install_neuronx_cc_hook()
        assert nc.dbg_addr is None
        pid = nc.partition_id_tensor
        self.partition_name = pid.name if pid else None
        in_names, out_names, out_avals = [], [], []
        for alloc in nc.m.functions[0].allocations:
            if not isinstance(alloc, mybir.MemoryLocationSet):
                continue
            name = alloc.memorylocations[0].name
            if alloc.kind == "ExternalInput":
                if name != self.partition_name:
                    in_names.append(name)
            elif alloc.kind == "ExternalOutput":
                shape = tuple(alloc.tensor_shape)
                dtype = mybir.dt.np(alloc.dtype)
                out_names.append(name)
                out_avals.append(jax.core.ShapedArray(shape, dtype))
        all_names = tuple(in_names + out_names +
                          ([self.partition_name] if self.partition_name else []))

        def _body(*args):
            operands = list(args)
            if self.partition_name is not None:
                operands.append(bass2jax.partition_id_tensor())
            return tuple(bass2jax._bass_exec_p.bind(
                *operands, out_avals=tuple(out_avals), in_names=all_names,
                out_names=tuple(out_names), lowering_input_output_aliases=(),
                sim_require_finite=True, sim_require_nnan=True, nc=nc))

        devices = jax.devices()[:NCORES]
        self.mesh = Mesh(np.asarray(devices), ("core",))
        self.sharding = NamedSharding(self.mesh, PartitionSpec("core"))
        n_in, n_out = len(in_names), len(out_names)
        self.fn = jax.jit(
            shard_map(_body, mesh=self.mesh,
                      in_specs=(PartitionSpec("core"),) * (n_in + n_out),
                      out_specs=(PartitionSpec("core"),) * n_out,
                      check_rep=False),
            donate_argnums=tuple(range(n_in, n_in + n_out)),
            keep_unused=True)
        self.in_names = in_names
        self.out_avals = out_avals

    def put(self, a):
        return jax.device_put(np.ascontiguousarray(a), self.sharding)

    def rep(self, a):
        """Replicate a per-core operand into the global (8*rows, ...) layout."""
        a = np.ascontiguousarray(a)
        g = np.broadcast_to(a[None], (NCORES,) + a.shape)
        return self.put(g.reshape(NCORES * a.shape[0], *a.shape[1:]))

    def run(self, by_name):
        args = [by_name[n] for n in self.in_names]
        zeros = [np.zeros((NCORES * av.shape[0], *av.shape[1:]), av.dtype)
                 for av in self.out_avals]
        outs = self.fn(*args, *zeros)
        return np.asarray(outs[0])


_STATE = None


def _init_state():
    bf = ml_dtypes.bfloat16
    nc = _build_nc()
    r = _Runner(nc)

    selbc = np.zeros((RH, 32, 128), bf)
    for c in range(32):
        selbc[2 * c, c, :64] = 1
        selbc[2 * c + 1, c, 64:] = 1
    hred = np.zeros((128, H), np.float32)
    for h in range(H):
        hred[h * 32:(h + 1) * 32, h] = 1
    hexp = np.ascontiguousarray(hred.T)
    iota2 = (np.arange(128, dtype=np.float32)[:, None]
             + 128.0 * np.arange(NCH, dtype=np.float32)[None, :])
    # center index row differs per core: within-batch node id repeated K times
    gcb = np.empty((NCORES, E), np.float32)
    for c in range(NCORES):
        loc = (c % (NCORES // B)) * PCORE + np.arange(PCORE)
        gcb[c] = np.repeat(loc, K).astype(np.float32)

    const = dict(
        selbc=r.rep(selbc), hred=r.rep(hred), hexp=r.rep(hexp),
        iota2=r.rep(iota2), gcb=r.put(gcb),
        ones64=r.rep(np.ones((D, 1), np.float32)),
        od64=r.rep(np.full((1, D), 1.0 / RH, np.float32)),
        ones1x64=r.rep(np.ones((1, D), np.float32)),
        ones1x128=r.rep(np.ones((1, 128), np.float32)),
        ones1x4=r.rep(np.ones((1, H), np.float32)),
    )
    return {"runner": r, "const": const, "wkey": None, "wdev": None}


def _weights_key(ws):
    h = hashlib.blake2b(digest_size=16)
    for a in ws:
        a = np.asarray(a)
        h.update(str(a.shape).encode()); h.update(str(a.dtype).encode())
        h.update(np.ascontiguousarray(a).tobytes())
    return h.digest()


def _prep_weights(r, norm_scale, Wq, Wxi, Wxj, rp_W1, rp_b1, rp_g1,
                  rp_W2, rp_b2, rp_g2, rp_W3, rp_b3, Wkv_out, Wout):
    bf = ml_dtypes.bfloat16
    WxjI = np.concatenate([np.asarray(Wxj, np.float32),
                           np.eye(D, dtype=np.float32)], axis=1)
    W3sb = np.ascontiguousarray(
        np.asarray(rp_W3, np.float32)
        .reshape(RH, KVD, D).transpose(0, 2, 1)       # (r, d, o)
        .reshape(RH * D, KVD)                         # row = r*64 + d
        .reshape(32, 128, KVD).transpose(1, 0, 2)     # (p, chunk, o)
    ).astype(bf)
    b3T = np.ascontiguousarray(
        np.asarray(rp_b3, np.float32).reshape(KVD, D).T).astype(bf)
    WkvP = np.ascontiguousarray(
        np.asarray(Wkv_out, np.float32).reshape(2, 128, KVD).transpose(1, 0, 2))
    return dict(
        nsc=r.rep(np.asarray(norm_scale, np.float32).reshape(D, 1)),
        Wq=r.rep(np.asarray(Wq, np.float32)),
        Wxi=r.rep(np.asarray(Wxi, np.float32)),
        WxjI=r.rep(WxjI),
        W1=r.rep(np.asarray(rp_W1, np.float32).reshape(1, RH)),
        b1=r.rep(np.asarray(rp_b1, np.float32).reshape(RH, 1)),
        g1=r.rep(np.asarray(rp_g1, np.float32).reshape(RH, 1)),
        W2=r.rep(np.asarray(rp_W2, np.float32)),
        b2=r.rep(np.asarray(rp_b2, np.float32).reshape(RH, 1)),
        g2=r.rep(np.asarray(rp_g2, np.float32).reshape(RH, 1)),
        W3sb=r.rep(W3sb), b3T=r.rep(b3T), Wkv=r.rep(WkvP),
        Wout=r.rep(np.asarray(Wout, np.float32)),
    )


def kernel(features, neighbor_indices, neighbor_mask, rel_dist, norm_scale,
           Wq, Wxi, Wxj, rp_W1, rp_b1, rp_g1, rp_W2, rp_b2, rp_g2,
           rp_W3, rp_b3, Wkv_out, Wout):
    global _STATE
    if _STATE is None:
        _STATE = _init_state()
    st = _STATE
    r = st["runner"]

    wlist = (norm_scale, Wq, Wxi, Wxj, rp_W1, rp_b1, rp_g1, rp_W2, rp_b2,
             rp_g2, rp_W3, rp_b3, Wkv_out, Wout)
    wkey = _weights_key(wlist)
    if st["wkey"] != wkey:
        st["wdev"] = _prep_weights(r, *wlist)
        st["wkey"] = wkey

    f = np.asarray(features, np.float32)
    idx = np.asarray(neighbor_indices)
    msk = np.asarray(neighbor_mask)
    rd = np.asarray(rel_dist, np.float32)

    # per-core activations, laid out as the global (8*rows, ...) arrays
    fb = np.ascontiguousarray(f[..., 0].transpose(0, 2, 1))       # (B, D, N)
    fT = np.broadcast_to(fb[:, None], (B, NCORES // B, D, NB)) \
           .reshape(NCORES * D, NB)                               # core c -> batch c//4
    gnb = idx.reshape(NCORES, E).astype(np.float32)
    rdT = rd.reshape(NCORES, E)
    M1 = msk.reshape(NCORES, E).astype(np.float32)

    by_name = dict(st["const"])
    by_name.update(st["wdev"])
    by_name.update(fT=np.ascontiguousarray(fT), gnb=gnb,
                   rdT=np.ascontiguousarray(rdT), M1=M1)
    out = r.run(by_name)                                          # (8*PCORE, D)
    return out.reshape(B, N, D, 1).astype(np.float32)
